# revision 1
# baseline (speedup 1.0000x reference)
"""GCN2Net Trainium2 kernel (8-core SPMD).

Strategy:
- Host: fold gcn-norm into separable per-node scales (dinv), dedupe edges,
  renumber nodes (greedy bin-pack by in-degree) so every core has 98 dst-tiles
  with near-equal edge counts; pad per-(tile,segment) edge cells to 5 chunks of
  128. Self-loops handled separately via a diagonal matmul.
- Device (per core): initial projection (PE), then 8 layers of:
  batched fp16 dma_gather of x_scaled rows -> one-hot segment-matrix (fused
  is_equal*scale on DVE) -> PE matmul accumulation in PSUM (transposed
  activations [hid, node]) -> residual + identity-mapped dense matmul -> relu
  -> PE transpose -> dinv-scaled fp16 write -> AllGather across 8 cores.
- Final projection on device; host reassembles the node permutation.
"""
import math
import os
import numpy as np

P = 128
N_CORES = 8
N_NODES = 100000
IN_DIM = 512
HID = 128
N_LAYERS = 8
ALPHA = 0.1
THETA = 0.5

T_PER_CORE = 98
SHARD = T_PER_CORE * P          # 12544
NSEG = 4
QROWS = SHARD // NSEG           # 3136 rows per core per segment
SEG_ROWS = N_CORES * QROWS      # 25088 (< 32768, int16-addressable)
NROWS = N_CORES * SHARD         # 100352
CH_PER_CELL = 5
CAP = CH_PER_CELL * P           # 640
CH_PER_TILE = NSEG * CH_PER_CELL  # 20
G_TILES = 7                     # tiles per gather group
N_GROUPS = T_PER_CORE // G_TILES  # 14
GCH = G_TILES * CH_PER_CELL     # 35 chunks per (group, seg) gather
GIDX = GCH * P                  # 4480 indices per gather call

BETAS = [math.log(THETA / (i + 1) + 1.0) for i in range(N_LAYERS)]


# ----------------------------------------------------------------- host prep
def _preprocess(x, edge_index):
    import heapq

    src = np.asarray(edge_index[0], dtype=np.int64)
    dst = np.asarray(edge_index[1], dtype=np.int64)

    deg = np.bincount(dst, minlength=N_NODES).astype(np.float64) + 1.0
    dinv = (1.0 / np.sqrt(deg)).astype(np.float32)

    key = src * N_NODES + dst
    uk, counts = np.unique(key, return_counts=True)
    usrc = (uk // N_NODES).astype(np.int64)
    udst = (uk % N_NODES).astype(np.int64)
    is_self = usrc == udst
    selfw = np.ones(N_NODES, dtype=np.float32)
    selfw[udst[is_self]] += counts[is_self]
    usrc, udst = usrc[~is_self], udst[~is_self]
    um = counts[~is_self].astype(np.float32)

    din = np.bincount(udst, minlength=N_NODES)

    n_tiles_total = N_CORES * T_PER_CORE
    order = np.argsort(-din, kind="stable")
    heap = [(0, t) for t in range(n_tiles_total)]
    heapq.heapify(heap)
    tile_of = np.empty(N_NODES, dtype=np.int32)
    slot_of = np.empty(N_NODES, dtype=np.int32)
    tile_fill = np.zeros(n_tiles_total, dtype=np.int32)
    tile_load = np.zeros(n_tiles_total, dtype=np.int64)
    for v in order:
        while True:
            load, t = heapq.heappop(heap)
            if tile_fill[t] < P:
                break
        tile_of[v] = t
        slot_of[v] = tile_fill[t]
        tile_fill[t] += 1
        tile_load[t] = load + din[v]
        if tile_fill[t] < P:
            heapq.heappush(heap, (tile_load[t], t))

    core_of = (tile_of // T_PER_CORE).astype(np.int32)
    ppos = (tile_of.astype(np.int64) % T_PER_CORE) * P + slot_of   # pos within core
    seg_of = ppos // QROWS                                          # quarter
    srow = core_of.astype(np.int64) * QROWS + ppos % QROWS          # row in segment

    e_cell = (tile_of[udst].astype(np.int64) * NSEG + seg_of[usrc])
    cell_counts = np.bincount(e_cell, minlength=n_tiles_total * NSEG)
    assert cell_counts.max() <= CAP, f"cell overflow {cell_counts.max()}"
    order_e = np.argsort(e_cell, kind="stable")
    usrc, udst, um, e_cell = usrc[order_e], udst[order_e], um[order_e], e_cell[order_e]
    cell_starts = np.zeros(n_tiles_total * NSEG + 1, dtype=np.int64)
    np.cumsum(cell_counts, out=cell_starts[1:])

    xT = np.ascontiguousarray(np.asarray(x, dtype=np.float32).T)

    per_core = []
    for c in range(N_CORES):
        # slot-level edge arrays, [tile][seg][cap]
        gidx = np.zeros((T_PER_CORE, NSEG, CAP), dtype=np.int16)
        sscl = np.zeros((T_PER_CORE, NSEG, CAP), dtype=np.float32)
        drel = np.zeros((T_PER_CORE, NSEG, CAP), dtype=np.float32)
        base = (c * T_PER_CORE) * NSEG
        for t in range(T_PER_CORE):
            for s in range(NSEG):
                cid = base + t * NSEG + s
                a, b = cell_starts[cid], cell_starts[cid + 1]
                n = b - a
                if n:
                    gidx[t, s, :n] = srow[usrc[a:b]].astype(np.int16)
                    sscl[t, s, :n] = 0.9 * um[a:b] * dinv[udst[a:b]]
                    drel[t, s, :n] = slot_of[udst[a:b]].astype(np.float32)

        # gather-idx stream: [group][seg] blocks of GIDX indices, 16-wrapped+replicated
        gidx_w = np.zeros((P, N_GROUPS * NSEG * (GIDX // 16)), dtype=np.int16)
        for g in range(N_GROUPS):
            for s in range(NSEG):
                blk = g * NSEG + s
                stream = gidx[g * G_TILES:(g + 1) * G_TILES, s, :].reshape(GIDX)
                wrap = stream.reshape(GIDX // 16, 16).T      # [16, GIDX/16]
                col0 = blk * (GIDX // 16)
                for rg in range(8):
                    gidx_w[rg * 16:(rg + 1) * 16, col0:col0 + GIDX // 16] = wrap

        # chunk-column layout [P, T*CH_PER_TILE]: col t*20 + s*5 + cc
        sscl_cols = sscl.reshape(T_PER_CORE, NSEG * CH_PER_CELL, P).transpose(2, 0, 1).reshape(P, -1)
        drel_cols = drel.reshape(T_PER_CORE, NSEG * CH_PER_CELL, P).transpose(2, 0, 1).reshape(P, -1)

        mask = core_of == c
        vids = np.nonzero(mask)[0]
        pos = (tile_of[vids] % T_PER_CORE).astype(np.int64) * P + slot_of[vids]
        x_shard_T = np.zeros((IN_DIM, SHARD), dtype=np.float32)
        x_shard_T[:, pos] = xT[:, vids]
        dinv_col = np.zeros((P, T_PER_CORE), dtype=np.float32)
        dinvp_col = np.zeros((P, T_PER_CORE), dtype=np.float32)
        selfscale = np.zeros((P, T_PER_CORE), dtype=np.float32)
        sl = slot_of[vids]
        tl = tile_of[vids] % T_PER_CORE
        dinv_col[sl, tl] = dinv[vids]
        dinvp_col[sl, tl] = dinv[vids] / ALPHA
        selfscale[sl, tl] = 0.9 * dinv[vids] * selfw[vids]

        per_core.append(dict(
            x_shard_T=x_shard_T,
            gidx=gidx_w,
            sscl=np.ascontiguousarray(sscl_cols),
            drel=np.ascontiguousarray(drel_cols.astype(np.float16)),
            dinv_col=dinv_col,
            dinvp_col=dinvp_col,
            selfscale=selfscale,
        ))
    return per_core, core_of, tile_of, slot_of


# ------------------------------------------------------------- device kernel
_BUILD_CACHE = {}


def _build(n_layers=N_LAYERS):
    key = n_layers
    if key in _BUILD_CACHE:
        return _BUILD_CACHE[key]
    import concourse.bass as bass
    import concourse.bacc as bacc
    import concourse.tile as tile
    import concourse.mybir as mybir

    F32 = mybir.dt.float32
    F16 = mybir.dt.float16
    I16 = mybir.dt.int16
    AT = mybir.AluOpType
    ts = bass.ts

    nc = bacc.Bacc("TRN2", target_bir_lowering=False, debug=False,
                   num_devices=N_CORES)

    # inputs
    x_in = nc.dram_tensor("x_shard_T", [IN_DIM, SHARD], F32, kind="ExternalInput")
    gidx_in = nc.dram_tensor("gidx", [P, N_GROUPS * NSEG * (GIDX // 16)], I16, kind="ExternalInput")
    sscl_in = nc.dram_tensor("sscl", [P, T_PER_CORE * CH_PER_TILE], F32, kind="ExternalInput")
    drel_in = nc.dram_tensor("drel", [P, T_PER_CORE * CH_PER_TILE], F16, kind="ExternalInput")
    dinv_in = nc.dram_tensor("dinv_col", [P, T_PER_CORE], F32, kind="ExternalInput")
    dinvp_in = nc.dram_tensor("dinvp_col", [P, T_PER_CORE], F32, kind="ExternalInput")
    selfs_in = nc.dram_tensor("selfscale", [P, T_PER_CORE], F32, kind="ExternalInput")
    iota_in = nc.dram_tensor("iota_row", [P, P], F16, kind="ExternalInput")
    iotac_in = nc.dram_tensor("iota_colp", [P, 1], F16, kind="ExternalInput")
    ident_in = nc.dram_tensor("ident", [P, P], F16, kind="ExternalInput")
    win_in = nc.dram_tensor("W_in_stack", [P, IN_DIM], F32, kind="ExternalInput")
    bin_in = nc.dram_tensor("b_in_col", [P, 1], F32, kind="ExternalInput")
    wl_in = nc.dram_tensor("Wl_stack", [P, n_layers * HID], F32, kind="ExternalInput")
    wout_in = nc.dram_tensor("W_out_col", [P, 1], F32, kind="ExternalInput")
    bout_in = nc.dram_tensor("b_out_s", [1, 1], F32, kind="ExternalInput")

    out_t = nc.dram_tensor("out_shard", [1, SHARD], F32, kind="ExternalOutput")

    with tile.TileContext(nc) as tc:
        with (
            tc.tile_pool(name="res", bufs=1) as res,            # resident SBUF
            tc.tile_pool(name="gpool", bufs=2) as gpool,        # gather bufs
            tc.tile_pool(name="work", bufs=2) as work,          # per-tile work
            tc.tile_pool(name="spool", bufs=3) as spool,        # S matrices
            tc.tile_pool(name="ppool_a", bufs=2, space="PSUM") as ppool_a,
            tc.tile_pool(name="ppool_b", bufs=2, space="PSUM") as ppool_b,
            tc.tile_pool(name="ppool_c", bufs=2, space="PSUM") as ppool_c,
            tc.tile_pool(name="dram", bufs=1, space="DRAM") as dram,
        ):
            # ---- resident loads
            sscl_r = res.tile([P, T_PER_CORE * CH_PER_TILE], F32)
            drel_r = res.tile([P, T_PER_CORE * CH_PER_TILE], F16)
            dinv_r = res.tile([P, T_PER_CORE], F32)
            dinvp_r = res.tile([P, T_PER_CORE], F32)
            selfs_r = res.tile([P, T_PER_CORE], F32)
            iota_r = res.tile([P, P], F16)
            iotac_r = res.tile([P, 1], F16)
            ident_r = res.tile([P, P], F16)
            win_r = res.tile([P, IN_DIM], F32)
            bin_r = res.tile([P, 1], F32)
            wl_r = res.tile([P, n_layers * HID], F32)
            wout_r = res.tile([P, 1], F32)
            bout_r = res.tile([1, 1], F32)
            x0s_r = res.tile([P, SHARD], F32)                   # 0.1*x0, [hid, node]
            orow_r = res.tile([1, SHARD], F32)
            identf_r = res.tile([P, P], F32)

            for sb, dr in [(sscl_r, sscl_in), (drel_r, drel_in), (dinv_r, dinv_in),
                           (dinvp_r, dinvp_in), (selfs_r, selfs_in), (iota_r, iota_in),
                           (iotac_r, iotac_in), (ident_r, ident_in), (win_r, win_in),
                           (bin_r, bin_in), (wl_r, wl_in), (wout_r, wout_in),
                           (bout_r, bout_in)]:
                nc.sync.dma_start(sb[:], dr[:])
            nc.vector.tensor_copy(identf_r[:], ident_r[:])

            # ---- DRAM buffers
            xnext = dram.tile([SHARD, HID], F16)                # own scaled shard
            xf = [[dram.tile([SEG_ROWS, HID], F16, addr_space="Shared",
                             name=f"xf{i}_{s}") for s in range(NSEG)]
                  for i in range(n_layers)]

            # ---- initial projection
            for t in range(T_PER_CORE):
                xt = work.tile([P, IN_DIM], F32, name="xt")
                for k in range(IN_DIM // P):
                    nc.sync.dma_start(xt[:, ts(k, P)], x_in[ts(k, P), ts(t, P)])
                ps_x = ppool_a.tile([P, P], F32, name="ps_x", tag="ps_agg")
                for k in range(IN_DIM // P):
                    nc.tensor.matmul(
                        out=ps_x[:], lhsT=win_r[:, ts(k, P)], rhs=xt[:, ts(k, P)],
                        start=(k == 0), stop=(k == IN_DIM // P - 1))
                # x0s = (psum + b) * alpha  -> resident
                nc.vector.tensor_scalar(
                    out=x0s_r[:, ts(t, P)], in0=ps_x[:],
                    scalar1=bin_r[:], scalar2=ALPHA,
                    op0=AT.add, op1=AT.mult)
                # transpose (f32) and scale by dinv/alpha -> xnext f16
                ps_t = ppool_c.tile([P, P], F32, name="ps_t", tag="ps_t2")
                nc.tensor.matmul(out=ps_t[:], lhsT=x0s_r[:, ts(t, P)], rhs=identf_r[:],
                                 is_transpose=True)
                xn_sb = work.tile([P, P], F16, name="xn_sb")
                nc.vector.tensor_scalar(
                    out=xn_sb[:], in0=ps_t[:], scalar1=dinvp_r[:, t:t + 1],
                    scalar2=None, op0=AT.mult)
                nc.sync.dma_start(xnext[ts(t, P), :], xn_sb[:])

            for s in range(NSEG):
                nc.gpsimd.collective_compute(
                    "AllGather", mybir.AluOpType.bypass,
                    replica_groups=[list(range(N_CORES))],
                    ins=[xnext[s * QROWS:(s + 1) * QROWS, :]],
                    outs=[xf[0][s].opt()])

            # ---- layers
            for l in range(n_layers):
                beta = BETAS[l]
                xsrc = xf[l]
                for g in range(N_GROUPS):
                    gb = []
                    for s in range(NSEG):
                        blk = g * NSEG + s
                        gi = work.tile([P, GIDX // 16], I16, name=f"gi{s}")
                        nc.sync.dma_start(
                            gi[:], gidx_in[:, blk * (GIDX // 16):(blk + 1) * (GIDX // 16)])
                        gbuf = gpool.tile([P, GCH * P], F16, name=f"gbuf{s}")
                        nc.gpsimd.dma_gather(
                            out_ap=gbuf[:].rearrange("p (c e) -> p c e", c=GCH),
                            in_ap=xsrc[s][:],
                            idxs_ap=gi[:],
                            num_idxs=GIDX, num_idxs_reg=GIDX, elem_size=HID,
                            single_packet=False)
                        gb.append(gbuf)
                    for tt in range(G_TILES):
                        t = g * G_TILES + tt
                        ps_agg = ppool_a.tile([P, P], F32, name="ps_agg")
                        for s in range(NSEG):
                            for cc in range(CH_PER_CELL):
                                ch = s * CH_PER_CELL + cc
                                col = t * CH_PER_TILE + ch
                                s_t = spool.tile([P, P], F16, name="s_t")
                                nc.vector.scalar_tensor_tensor(
                                    out=s_t[:], in0=iota_r[:],
                                    scalar=drel_r[:, col:col + 1],
                                    in1=sscl_r[:, col:col + 1].to_broadcast([P, P]),
                                    op0=AT.is_equal, op1=AT.mult)
                                nc.tensor.matmul(
                                    out=ps_agg[:],
                                    lhsT=gb[s][:, ts(tt * CH_PER_CELL + cc, P)],
                                    rhs=s_t[:],
                                    start=(ch == 0), stop=False)
                        # self-loop: xnext rows are last layer's scaled x
                        xself = work.tile([P, P], F16, name="xself")
                        nc.sync.dma_start(xself[:], xnext[ts(t, P), :])
                        diag = spool.tile([P, P], F16, name="diag")
                        nc.vector.scalar_tensor_tensor(
                            out=diag[:], in0=iota_r[:], scalar=iotac_r[:],
                            in1=selfs_r[:, t:t + 1].to_broadcast([P, P]),
                            op0=AT.is_equal, op1=AT.mult)
                        nc.tensor.matmul(out=ps_agg[:], lhsT=xself[:], rhs=diag[:],
                                         start=False, stop=True)
                        # h = agg + x0s
                        h_t = work.tile([P, P], F32, name="h_t")
                        nc.vector.tensor_tensor(
                            out=h_t[:], in0=ps_agg[:], in1=x0s_r[:, ts(t, P)], op=AT.add)
                        # dense: z = W'_l.T-free form -> psum
                        ps_d = ppool_b.tile([P, P], F32, name="ps_d")
                        nc.tensor.matmul(out=ps_d[:], lhsT=wl_r[:, ts(l, P)], rhs=h_t[:],
                                         start=True, stop=True)
                        if l < n_layers - 1:
                            sum_t = work.tile([P, P], F32, name="sum_t")
                            nc.vector.tensor_tensor(
                                out=sum_t[:], in0=ps_d[:], in1=h_t[:], op=AT.add)
                            xn_t = work.tile([P, P], F16, name="xn_t")
                            nc.scalar.activation(
                                xn_t[:], sum_t[:],
                                mybir.ActivationFunctionType.Relu, scale=1.0 - beta)
                            ps_t2 = ppool_c.tile([P, P], F16, name="ps_t2", tag="ps_t2")
                            nc.tensor.matmul(out=ps_t2[:], lhsT=xn_t[:], rhs=ident_r[:],
                                             is_transpose=True)
                            xn_sb2 = work.tile([P, P], F16, name="xn_sb2")
                            nc.vector.tensor_scalar(
                                out=xn_sb2[:], in0=ps_t2[:],
                                scalar1=dinv_r[:, t:t + 1], scalar2=None, op0=AT.mult)
                            nc.sync.dma_start(xnext[ts(t, P), :], xn_sb2[:])
                        else:
                            sum_t = work.tile([P, P], F32, name="sum_t")
                            nc.vector.tensor_tensor(
                                out=sum_t[:], in0=ps_d[:], in1=h_t[:], op=AT.add)
                            xn_f = work.tile([P, P], F32, name="xn_f")
                            nc.scalar.activation(
                                xn_f[:], sum_t[:],
                                mybir.ActivationFunctionType.Relu, scale=1.0 - beta)
                            ps_o = ppool_b.tile([1, P], F32, name="ps_o", tag="ps_d")
                            nc.tensor.matmul(out=ps_o[:], lhsT=wout_r[:], rhs=xn_f[:],
                                             start=True, stop=True)
                            nc.vector.tensor_scalar(
                                out=orow_r[:, ts(t, P)], in0=ps_o[:],
                                scalar1=bout_r[:], scalar2=None, op0=AT.add)
                if l < n_layers - 1:
                    for s in range(NSEG):
                        nc.gpsimd.collective_compute(
                            "AllGather", mybir.AluOpType.bypass,
                            replica_groups=[list(range(N_CORES))],
                            ins=[xnext[s * QROWS:(s + 1) * QROWS, :]],
                            outs=[xf[l + 1][s].opt()])

            nc.sync.dma_start(out_t[:], orow_r[:])

    nc.compile()
    _BUILD_CACHE[key] = nc
    return nc


# ------------------------------------------------------------------ runner
def kernel(x, edge_index, edge_weight, W_in, b_in, W_layers, W_out, b_out):
    import concourse.bass_utils as bass_utils

    x = np.asarray(x)
    per_core, core_of, tile_of, slot_of = _preprocess(x, edge_index)

    W_in = np.asarray(W_in, np.float32)
    b_in = np.asarray(b_in, np.float32)
    W_layers = np.asarray(W_layers, np.float32)
    W_out = np.asarray(W_out, np.float32)
    b_out = np.asarray(b_out, np.float32)

    win_stack = W_in.reshape(IN_DIM // P, P, HID).transpose(1, 0, 2).reshape(P, IN_DIM)
    wl_stack = np.concatenate(
        [BETAS[l] / (1.0 - BETAS[l]) * W_layers[l] for l in range(N_LAYERS)],
        axis=1)                                    # [128, 8*128]
    iota_row = np.broadcast_to(np.arange(P, dtype=np.float32), (P, P)).astype(np.float16)
    iota_colp = np.arange(P, dtype=np.float32).reshape(P, 1).astype(np.float16)
    ident = np.eye(P, dtype=np.float16)

    in_maps = []
    for c in range(N_CORES):
        d = per_core[c]
        in_maps.append({
            "x_shard_T": d["x_shard_T"],
            "gidx": d["gidx"],
            "sscl": d["sscl"],
            "drel": d["drel"].astype(np.float16),
            "dinv_col": d["dinv_col"],
            "dinvp_col": d["dinvp_col"],
            "selfscale": d["selfscale"],
            "iota_row": iota_row,
            "iota_colp": iota_colp,
            "ident": ident,
            "W_in_stack": np.ascontiguousarray(win_stack),
            "b_in_col": b_in.reshape(P, 1),
            "Wl_stack": np.ascontiguousarray(wl_stack),
            "W_out_col": W_out.reshape(P, 1),
            "b_out_s": b_out.reshape(1, 1),
        })

    nc = _build(int(os.environ.get('GCN_LAYERS', str(N_LAYERS))))
    trace = bool(int(os.environ.get("GCN_TRACE", "0")))
    res = bass_utils.run_bass_kernel_spmd(
        nc, in_maps, core_ids=list(range(N_CORES)), trace=trace)
    kernel.last_results = res

    out = np.zeros((N_NODES, 1), dtype=np.float32)
    pos = (tile_of % T_PER_CORE).astype(np.int64) * P + slot_of
    for c in range(N_CORES):
        mask = core_of == c
        out[mask, 0] = res.results[c]["out_shard"][0, pos[mask]]
    return out



# revision 5
# speedup vs baseline: 1.0617x; 1.0617x over previous
"""GCN2Net Trainium2 kernel (8-core SPMD).

Strategy:
- Host: fold the full gcn-norm (dinv[src]*dinv[dst]), the 0.9/0.1 residual
  scales, and the per-layer identity-mapping ((1-beta)I + beta*W) into edge
  scales / weight stacks.  Dedupe edges, renumber nodes (greedy bin-pack by
  in-degree) so every core owns 98 dst-tiles of 128 nodes with near-equal
  edge counts; per-(tile, src-segment) edge cells padded to 5 chunks of 128.
- Device (per core): bf16 initial projection (PE), then 8 layers of:
  batched fp16 dma_gather of x rows -> one-hot S-matrix built on DVE via
  single-src tensor_scalar (is_equal x mult, 4x mode) -> PE matmul
  accumulation in PSUM (transposed activations [hid, node]) -> self-loop via
  diagonal matmul on the SBUF-resident previous tile -> initial-residual via
  identity matmul -> PSUM->SBUF on ACT -> dense W' matmul -> relu (ACT) ->
  PE transpose -> SBUF (resident xprev) -> DMA to xnext -> per-segment
  AllGather pipelined under the remaining tiles' compute.
- Final projection on device; host reassembles the node permutation.
"""
import math
import os
import numpy as np

P = 128
N_CORES = 8
N_NODES = 100000
IN_DIM = 512
HID = 128
N_LAYERS = 8
ALPHA = 0.1
THETA = 0.5

T_PER_CORE = 98
SHARD = T_PER_CORE * P          # 12544
NSEG = 4
QROWS = SHARD // NSEG           # 3136 rows per core per segment
SEG_ROWS = N_CORES * QROWS      # 25088 (< 32768, int16-addressable)
NROWS = N_CORES * SHARD         # 100352
CH_PER_CELL = 5
CAP = CH_PER_CELL * P           # 640
CH_PER_TILE = NSEG * CH_PER_CELL  # 20
G_TILES = 7                     # tiles per gather group
N_GROUPS = T_PER_CORE // G_TILES  # 14
GCH = G_TILES * CH_PER_CELL     # 35 chunks per (group, seg) gather
GIDX = GCH * P                  # 4480 indices per gather call

BETAS = [math.log(THETA / (i + 1) + 1.0) for i in range(N_LAYERS)]

GATHER_MODE = int(os.environ.get("GCN_GATHER_MODE", "0"))  # 0=direct, 1=prep+trigger
N_QUEUES = int(os.environ.get("GCN_QUEUES", "1"))


# ----------------------------------------------------------------- host prep
def _preprocess(x, edge_index):
    import heapq

    src = np.asarray(edge_index[0], dtype=np.int64)
    dst = np.asarray(edge_index[1], dtype=np.int64)

    deg = np.bincount(dst, minlength=N_NODES).astype(np.float64) + 1.0
    dinv = (1.0 / np.sqrt(deg)).astype(np.float64)

    key = src * N_NODES + dst
    uk, counts = np.unique(key, return_counts=True)
    usrc = (uk // N_NODES).astype(np.int64)
    udst = (uk % N_NODES).astype(np.int64)
    is_self = usrc == udst
    selfw = np.ones(N_NODES, dtype=np.float64)
    selfw[udst[is_self]] += counts[is_self]
    usrc, udst = usrc[~is_self], udst[~is_self]
    um = counts[~is_self].astype(np.float64)

    din = np.bincount(udst, minlength=N_NODES)

    n_tiles_total = N_CORES * T_PER_CORE
    order = np.argsort(-din, kind="stable")
    heap = [(0, t) for t in range(n_tiles_total)]
    heapq.heapify(heap)
    tile_of = np.empty(N_NODES, dtype=np.int32)
    slot_of = np.empty(N_NODES, dtype=np.int32)
    tile_fill = np.zeros(n_tiles_total, dtype=np.int32)
    tile_load = np.zeros(n_tiles_total, dtype=np.int64)
    for v in order:
        while True:
            load, t = heapq.heappop(heap)
            if tile_fill[t] < P:
                break
        tile_of[v] = t
        slot_of[v] = tile_fill[t]
        tile_fill[t] += 1
        tile_load[t] = load + din[v]
        if tile_fill[t] < P:
            heapq.heappush(heap, (tile_load[t], t))

    core_of = (tile_of // T_PER_CORE).astype(np.int32)
    ppos = (tile_of.astype(np.int64) % T_PER_CORE) * P + slot_of   # pos within core
    seg_of = ppos // QROWS                                          # quarter
    srow = core_of.astype(np.int64) * QROWS + ppos % QROWS          # row in segment

    # fully folded edge weight: 0.9 * mult * dinv_s * dinv_d
    ew = (0.9 * um * dinv[usrc] * dinv[udst]).astype(np.float32)
    selfsc = (0.9 * selfw * dinv * dinv).astype(np.float32)

    e_cell = (tile_of[udst].astype(np.int64) * NSEG + seg_of[usrc])
    cell_counts = np.bincount(e_cell, minlength=n_tiles_total * NSEG)
    assert cell_counts.max() <= CAP, f"cell overflow {cell_counts.max()}"
    # sort by cell, then by src row within cell (better HBM locality)
    order_e = np.lexsort((srow[usrc], e_cell))
    usrc, udst, ew, e_cell = usrc[order_e], udst[order_e], ew[order_e], e_cell[order_e]
    cell_starts = np.zeros(n_tiles_total * NSEG + 1, dtype=np.int64)
    np.cumsum(cell_counts, out=cell_starts[1:])

    xT = np.ascontiguousarray(np.asarray(x, dtype=np.float32).T)

    per_core = []
    for c in range(N_CORES):
        gidx = np.zeros((T_PER_CORE, NSEG, CAP), dtype=np.int16)
        sscl = np.zeros((T_PER_CORE, NSEG, CAP), dtype=np.float32)
        drel = np.zeros((T_PER_CORE, NSEG, CAP), dtype=np.float32)
        base = (c * T_PER_CORE) * NSEG
        for t in range(T_PER_CORE):
            for s in range(NSEG):
                cid = base + t * NSEG + s
                a, b = cell_starts[cid], cell_starts[cid + 1]
                n = b - a
                if n:
                    gidx[t, s, :n] = srow[usrc[a:b]].astype(np.int16)
                    sscl[t, s, :n] = ew[a:b]
                    drel[t, s, :n] = slot_of[udst[a:b]].astype(np.float32)

        # gather-idx stream: [group][seg] blocks of GIDX indices, 16-wrapped+replicated
        gidx_w = np.zeros((P, N_GROUPS * NSEG * (GIDX // 16)), dtype=np.int16)
        for g in range(N_GROUPS):
            for s in range(NSEG):
                blk = g * NSEG + s
                stream = gidx[g * G_TILES:(g + 1) * G_TILES, s, :].reshape(GIDX)
                wrap = stream.reshape(GIDX // 16, 16).T      # [16, GIDX/16]
                col0 = blk * (GIDX // 16)
                for rg in range(8):
                    gidx_w[rg * 16:(rg + 1) * 16, col0:col0 + GIDX // 16] = wrap

        # chunk-column layout [P, T*CH_PER_TILE]: col t*20 + s*5 + cc
        sscl_cols = sscl.reshape(T_PER_CORE, NSEG * CH_PER_CELL, P).transpose(2, 0, 1).reshape(P, -1)
        drel_cols = drel.reshape(T_PER_CORE, NSEG * CH_PER_CELL, P).transpose(2, 0, 1).reshape(P, -1)

        mask = core_of == c
        vids = np.nonzero(mask)[0]
        pos = (tile_of[vids] % T_PER_CORE).astype(np.int64) * P + slot_of[vids]
        x_shard_T = np.zeros((IN_DIM, SHARD), dtype=np.float32)
        x_shard_T[:, pos] = xT[:, vids]
        selfscale = np.zeros((P, T_PER_CORE), dtype=np.float32)
        sl = slot_of[vids]
        tl = tile_of[vids] % T_PER_CORE
        selfscale[sl, tl] = selfsc[vids]

        per_core.append(dict(
            x_shard_T=np.ascontiguousarray(x_shard_T, dtype=np.float32).astype(np.dtype("bfloat16") if False else np.float32),
            gidx=gidx_w,
            sscl=np.ascontiguousarray(sscl_cols),
            drel=np.ascontiguousarray(drel_cols),
            selfscale=selfscale,
        ))
    return per_core, core_of, tile_of, slot_of


# ------------------------------------------------------------- device kernel
_BUILD_CACHE = {}


def _build(n_layers=N_LAYERS):
    key = (n_layers, GATHER_MODE, N_QUEUES)
    if key in _BUILD_CACHE:
        return _BUILD_CACHE[key]
    import concourse.bass as bass
    import concourse.bacc as bacc
    import concourse.tile as tile
    import concourse.mybir as mybir

    F32 = mybir.dt.float32
    F16 = mybir.dt.float16
    BF16 = mybir.dt.bfloat16
    I16 = mybir.dt.int16
    AT = mybir.AluOpType
    ts = bass.ts

    nc = bacc.Bacc("TRN2", target_bir_lowering=False, debug=False,
                   num_devices=N_CORES)

    # inputs
    x_in = nc.dram_tensor("x_shard_T", [IN_DIM, SHARD], BF16, kind="ExternalInput")
    gidx_in = nc.dram_tensor("gidx", [P, N_GROUPS * NSEG * (GIDX // 16)], I16, kind="ExternalInput")
    sscl_in = nc.dram_tensor("sscl", [P, T_PER_CORE * CH_PER_TILE], F32, kind="ExternalInput")
    drel_in = nc.dram_tensor("drel", [P, T_PER_CORE * CH_PER_TILE], F32, kind="ExternalInput")
    selfs_in = nc.dram_tensor("selfscale", [P, T_PER_CORE], F32, kind="ExternalInput")
    iota_in = nc.dram_tensor("iota_row", [P, P], F16, kind="ExternalInput")
    iotac_in = nc.dram_tensor("iota_colp", [P, 1], F32, kind="ExternalInput")
    ident_in = nc.dram_tensor("ident", [P, P], F16, kind="ExternalInput")
    win_in = nc.dram_tensor("W_in_stack", [P, IN_DIM], BF16, kind="ExternalInput")
    bin_in = nc.dram_tensor("b_in_col", [P, 1], F32, kind="ExternalInput")
    wl_in = nc.dram_tensor("Wl_stack", [P, n_layers * HID], F16, kind="ExternalInput")
    wout_in = nc.dram_tensor("W_out_col", [P, 1], F16, kind="ExternalInput")
    bout_in = nc.dram_tensor("b_out_s", [1, 1], F32, kind="ExternalInput")

    out_t = nc.dram_tensor("out_shard", [1, SHARD], F32, kind="ExternalOutput")

    with tile.TileContext(nc) as tc:
        with (
            tc.tile_pool(name="res", bufs=1) as res,            # resident SBUF
            tc.tile_pool(name="gpool", bufs=2) as gpool,        # gather bufs
            tc.tile_pool(name="work", bufs=3) as work,          # per-tile work
            tc.tile_pool(name="spool", bufs=4) as spool,        # S matrices
            tc.tile_pool(name="ppool_a", bufs=3, space="PSUM") as ppool_a,
            tc.tile_pool(name="ppool_b", bufs=2, space="PSUM") as ppool_b,
            tc.tile_pool(name="ppool_c", bufs=2, space="PSUM") as ppool_c,
            tc.tile_pool(name="dram", bufs=1, space="DRAM") as dram,
        ):
            # ---- resident loads
            sscl_r = res.tile([P, T_PER_CORE * CH_PER_TILE], F32)
            drel_r = res.tile([P, T_PER_CORE * CH_PER_TILE], F32)
            selfs_r = res.tile([P, T_PER_CORE], F32)
            iota_r = res.tile([P, P], F16)
            iotac_r = res.tile([P, 1], F32)
            ident_r = res.tile([P, P], F16)
            win_r = res.tile([P, IN_DIM], BF16)
            bin_r = res.tile([P, 1], F32)
            wl_r = res.tile([P, n_layers * HID], F16)
            wout_r = res.tile([P, 1], F16)
            bout_r = res.tile([1, 1], F32)
            x0s_r = res.tile([P, SHARD], F16)                   # 0.1*x0, [hid, node]
            xprev_r = res.tile([P, SHARD], F16)                 # prev layer, [node, hid] tiles
            orow_r = res.tile([1, SHARD], F32)

            for sb, dr in [(sscl_r, sscl_in), (drel_r, drel_in),
                           (selfs_r, selfs_in), (iota_r, iota_in),
                           (iotac_r, iotac_in), (ident_r, ident_in),
                           (win_r, win_in), (bin_r, bin_in), (wl_r, wl_in),
                           (wout_r, wout_in), (bout_r, bout_in)]:
                nc.sync.dma_start(sb[:], dr[:])

            # ---- DRAM buffers
            xnext = dram.tile([SHARD, HID], F16)                # own shard [node, hid]
            xf = [[dram.tile([SEG_ROWS, HID], F16, addr_space="Shared",
                             name=f"xf{i}_{s}") for s in range(NSEG)]
                  for i in range(n_layers)]

            if GATHER_MODE:
                gsems = [nc.alloc_semaphore(f"gq{q}") for q in range(N_QUEUES)]

            def gather(l, g, s, gbuf):
                blk = g * NSEG + s
                xsrc = xf[l][s]
                gi = work.tile([P, GIDX // 16], I16, name=f"gi{s}")
                nc.sync.dma_start(
                    gi[:], gidx_in[:, blk * (GIDX // 16):(blk + 1) * (GIDX // 16)])
                kw = dict(
                    out_ap=gbuf[:].rearrange("p (c e) -> p c e", c=GCH),
                    in_ap=xsrc[:],
                    idxs_ap=gi[:],
                    num_idxs=GIDX, num_idxs_reg=GIDX, elem_size=HID,
                    single_packet=False)
                if GATHER_MODE:
                    q = (l * N_GROUPS * NSEG + blk) % N_QUEUES
                    nc.gpsimd.dma_gather(prepare_only=True, sem=gsems[q],
                                         queue_num=q, **kw)
                    nc.gpsimd.trigger_dma(count=None, queue_num=q)
                else:
                    nc.gpsimd.dma_gather(**kw)

            # ---- initial projection (bf16)
            for t in range(T_PER_CORE):
                xt = work.tile([P, IN_DIM], BF16, name="xt")
                for k in range(IN_DIM // P):
                    nc.sync.dma_start(xt[:, ts(k, P)], x_in[ts(k, P), ts(t, P)])
                ps_x = ppool_a.tile([P, P], F32, name="ps_x", tag="ps_agg")
                for k in range(IN_DIM // P):
                    nc.tensor.matmul(
                        out=ps_x[:], lhsT=win_r[:, ts(k, P)], rhs=xt[:, ts(k, P)],
                        start=(k == 0), stop=(k == IN_DIM // P - 1))
                # x0s = (psum + b) * alpha  -> resident  [hid, node] f16
                nc.vector.tensor_scalar(
                    out=x0s_r[:, ts(t, P)], in0=ps_x[:],
                    scalar1=bin_r[:], scalar2=ALPHA,
                    op0=AT.add, op1=AT.mult)
                # xtil = psum + b -> f16 (scalar engine), then transpose -> xprev/xnext
                xtil = work.tile([P, P], F16, name="xtil")
                nc.vector.tensor_scalar(
                    out=xtil[:], in0=ps_x[:], scalar1=bin_r[:], scalar2=None,
                    op0=AT.add)
                ps_t = ppool_c.tile([P, P], F16, name="ps_t", tag="ps_t2")
                nc.tensor.matmul(out=ps_t[:], lhsT=xtil[:], rhs=ident_r[:],
                                 is_transpose=True)
                nc.vector.tensor_copy(xprev_r[:, ts(t, P)], ps_t[:])
                nc.sync.dma_start(xnext[ts(t, P), :], xprev_r[:, ts(t, P)])

            for s in range(NSEG):
                nc.gpsimd.collective_compute(
                    "AllGather", mybir.AluOpType.bypass,
                    replica_groups=[list(range(N_CORES))],
                    ins=[xnext[s * QROWS:(s + 1) * QROWS, :]],
                    outs=[xf[0][s].opt()])

            # ---- layers
            for l in range(n_layers):
                for g in range(N_GROUPS):
                    gb = []
                    for s in range(NSEG):
                        gbuf = gpool.tile([P, GCH * P], F16, name=f"gbuf{s}")
                        gather(l, g, s, gbuf)
                        gb.append(gbuf)
                    for tt in range(G_TILES):
                        t = g * G_TILES + tt
                        ps_agg = ppool_a.tile([P, P], F32, name="ps_agg")
                        first = True
                        for s in range(NSEG):
                            for cc in range(CH_PER_CELL):
                                col = t * CH_PER_TILE + s * CH_PER_CELL + cc
                                s_t = spool.tile([P, P], F16, name="s_t")
                                nc.vector.tensor_scalar(
                                    out=s_t[:], in0=iota_r[:],
                                    scalar1=drel_r[:, col:col + 1],
                                    scalar2=sscl_r[:, col:col + 1],
                                    op0=AT.is_equal, op1=AT.mult)
                                nc.tensor.matmul(
                                    out=ps_agg[:],
                                    lhsT=gb[s][:, ts(tt * CH_PER_CELL + cc, P)],
                                    rhs=s_t[:],
                                    start=first, stop=False)
                                first = False
                        # self-loop from resident xprev
                        diag = spool.tile([P, P], F16, name="diag")
                        nc.vector.tensor_scalar(
                            out=diag[:], in0=iota_r[:], scalar1=iotac_r[:],
                            scalar2=selfs_r[:, t:t + 1],
                            op0=AT.is_equal, op1=AT.mult)
                        nc.tensor.matmul(out=ps_agg[:], lhsT=xprev_r[:, ts(t, P)],
                                         rhs=diag[:], start=False, stop=False)
                        # initial residual: h += x0s (via identity matmul)
                        nc.tensor.matmul(out=ps_agg[:], lhsT=ident_r[:],
                                         rhs=x0s_r[:, ts(t, P)],
                                         start=False, stop=True)
                        # h -> SBUF f16 on scalar engine
                        h16 = work.tile([P, P], F16, name="h16")
                        nc.vector.tensor_copy(h16[:], ps_agg[:])
                        # dense: z = W'_l.T @ h
                        ps_d = ppool_b.tile([P, P], F32, name="ps_d")
                        nc.tensor.matmul(out=ps_d[:], lhsT=wl_r[:, ts(l, P)],
                                         rhs=h16[:], start=True, stop=True)
                        if l < n_layers - 1:
                            xn_t = work.tile([P, P], F16, name="xn_t")
                            nc.scalar.activation(
                                xn_t[:], ps_d[:],
                                mybir.ActivationFunctionType.Relu, scale=1.0)
                            ps_t2 = ppool_c.tile([P, P], F16, name="ps_t2", tag="ps_t2")
                            nc.tensor.matmul(out=ps_t2[:], lhsT=xn_t[:], rhs=ident_r[:],
                                             is_transpose=True)
                            nc.vector.tensor_copy(xprev_r[:, ts(t, P)], ps_t2[:])
                            nc.sync.dma_start(xnext[ts(t, P), :], xprev_r[:, ts(t, P)])
                        else:
                            xn_f = work.tile([P, P], F16, name="xn_f")
                            nc.scalar.activation(
                                xn_f[:], ps_d[:],
                                mybir.ActivationFunctionType.Relu, scale=1.0)
                            ps_o = ppool_b.tile([1, P], F32, name="ps_o", tag="ps_d")
                            nc.tensor.matmul(out=ps_o[:], lhsT=wout_r[:], rhs=xn_f[:],
                                             start=True, stop=True)
                            nc.vector.tensor_scalar(
                                out=orow_r[:, ts(t, P)], in0=ps_o[:],
                                scalar1=bout_r[:], scalar2=None, op0=AT.add)
                if l < n_layers - 1:
                    for s in range(NSEG):
                        nc.gpsimd.collective_compute(
                            "AllGather", mybir.AluOpType.bypass,
                            replica_groups=[list(range(N_CORES))],
                            ins=[xnext[s * QROWS:(s + 1) * QROWS, :]],
                            outs=[xf[l + 1][s].opt()])

            nc.sync.dma_start(out_t[:], orow_r[:])

    nc.compile()
    _BUILD_CACHE[key] = nc
    return nc


# ------------------------------------------------------------------ runner
def kernel(x, edge_index, edge_weight, W_in, b_in, W_layers, W_out, b_out):
    import concourse.bass_utils as bass_utils
    import ml_dtypes

    x = np.asarray(x)
    per_core, core_of, tile_of, slot_of = _preprocess(x, edge_index)

    W_in = np.asarray(W_in, np.float32)
    b_in = np.asarray(b_in, np.float32)
    W_layers = np.asarray(W_layers, np.float32)
    W_out = np.asarray(W_out, np.float32)
    b_out = np.asarray(b_out, np.float32)

    win_stack = W_in.reshape(IN_DIM // P, P, HID).transpose(1, 0, 2).reshape(P, IN_DIM)
    wl_stack = np.concatenate(
        [(1.0 - BETAS[l]) * np.eye(HID, dtype=np.float32) + BETAS[l] * W_layers[l]
         for l in range(N_LAYERS)], axis=1)        # [128, 8*128]
    iota_row = np.broadcast_to(np.arange(P, dtype=np.float32), (P, P)).astype(np.float16)
    iota_colp = np.arange(P, dtype=np.float32).reshape(P, 1)
    ident = np.eye(P, dtype=np.float16)

    in_maps = []
    for c in range(N_CORES):
        d = per_core[c]
        in_maps.append({
            "x_shard_T": d["x_shard_T"].astype(ml_dtypes.bfloat16),
            "gidx": d["gidx"],
            "sscl": d["sscl"],
            "drel": d["drel"],
            "selfscale": d["selfscale"],
            "iota_row": iota_row,
            "iota_colp": iota_colp,
            "ident": ident,
            "W_in_stack": np.ascontiguousarray(win_stack).astype(ml_dtypes.bfloat16),
            "b_in_col": b_in.reshape(P, 1),
            "Wl_stack": np.ascontiguousarray(wl_stack).astype(np.float16),
            "W_out_col": W_out.reshape(P, 1).astype(np.float16),
            "b_out_s": b_out.reshape(1, 1),
        })

    nc = _build(int(os.environ.get('GCN_LAYERS', str(N_LAYERS))))
    trace = bool(int(os.environ.get("GCN_TRACE", "0")))
    res = bass_utils.run_bass_kernel_spmd(
        nc, in_maps, core_ids=list(range(N_CORES)), trace=trace)
    kernel.last_results = res

    out = np.zeros((N_NODES, 1), dtype=np.float32)
    pos = (tile_of % T_PER_CORE).astype(np.int64) * P + slot_of
    for c in range(N_CORES):
        mask = core_of == c
        out[mask, 0] = res.results[c]["out_shard"][0, pos[mask]]
    return out
